# revision 1
# baseline (speedup 1.0000x reference)
"""Trainium2 Bass kernel for dilated 5x7 conv (128->16ch) + 1x1 (16->16) + 1x1 (16->128).

Strategy (data-parallel, 1 image per core across 8 cores):
  reference: y = conv_dilated(x, w3, dil=(2,3), pad=(4,9)); y = w4@y; y = w5@y
  Host folds w45 = w5 @ w4  [128, 16].

  Per core, image x [128, 56, 56] zero-padded to xp [128, 64, 74] (bf16):
  Stage 1 (TensorE): for each kw in 0..6, one matmul with
      lhsT = w1[:, kw, :] [c=128, (kh,co)=80], rhs = xp[:, rows, 3kw:3kw+56],
      PSUM-accumulating over kw  ->  P2[(kh,co), r, w] =
      sum_{kw,c} w3[co,c,kh,kw] * xp[c, r, w+3kw].
  Evacuate PSUM->SBUF with f32->bf16 cast (ScalarE).
  Shift-align (DMA, free-dim row offsets on same partitions):
      P2a[(kh,co), h, w] = P2[(kh,co), h+2kh, w]   for h in 0..55.
  Stage 2 (TensorE): out[o, h, w] = sum_{(kh,co)} w2[(kh,co), o] * P2a[(kh,co), h, w]
      -- a single K=80, M=128 matmul per 8-row chunk, w2[(kh,co), o] = w45[o, co].
  Evacuate (VectorE) and DMA out (f32).
"""

import os
import sys

import numpy as np

for _p in ("/opt/trn_rl_repo", "/root/.axon_site/_ro/trn_rl_repo"):
    if os.path.isdir(_p) and _p not in sys.path:
        sys.path.insert(0, _p)

import ml_dtypes  # noqa: E402

import concourse.bass as bass  # noqa: E402
import concourse.tile as tile  # noqa: E402
from concourse.tile_rust import add_dep_helper  # noqa: E402
from concourse import mybir  # noqa: E402
from concourse.bass_utils import run_bass_kernel_spmd  # noqa: E402

N, C, H, W = 8, 128, 56, 56
CO = 16
KH, KW = 5, 7
DH, DW = 2, 3
PH, PW = 4, 9
RP, WP = H + 2 * PH, W + 2 * PW  # 64 padded rows, 74 padded cols
M1 = KH * CO  # 80
RCH = 8  # stage-1 chunk: input rows per chunk
NCH1 = RP // RCH  # 8
OCH = 8  # stage-2 chunk: output rows per chunk
NCH2 = H // OCH  # 7
BF16 = mybir.dt.bfloat16
F32 = mybir.dt.float32

_NC = None


def _build_nc(attempt=0):
    nc = bass.Bass(
        "TRN2",
        target_bir_lowering=False,
        debug=False,
        enable_asserts=False,
        num_devices=N,
    )
    # all weights in ONE dram tensor/DMA so a single dummy matmul can absorb
    # the weight-DMA wait (the MM ISA slot fits only one semaphore wait).
    WKC = KW * M1 + KH * C  # 560 + 640
    xp_d = nc.dram_tensor("xp", [C, RP, WP], BF16, kind="ExternalInput")
    wk_d = nc.dram_tensor("wk", [C, WKC], BF16, kind="ExternalInput")
    out_d = nc.dram_tensor("out", [C, H * W], F32, kind="ExternalOutput")

    with tile.TileContext(nc) as tc:
        # schedule perturbation for compile-retry: the Tile scheduler is
        # process-state dependent and occasionally emits a tail Drain with
        # more semaphore waits than the ISA slot fits; a few extra leading
        # nops reshuffle the schedule.
        for _ in range(attempt):
            nc.sync.nop(nofuse=True)
        with (
            tc.tile_pool(name="const", bufs=1) as constp,
            tc.tile_pool(name="xin", bufs=1) as xinp,
            tc.tile_pool(name="p2s", bufs=1) as p2sp,
            tc.tile_pool(name="outs", bufs=1) as outsp,
            tc.tile_pool(name="psd", bufs=1, space="PSUM") as psd,
            tc.tile_pool(name="ps1", bufs=3, space="PSUM") as ps1,
            tc.tile_pool(name="ps2", bufs=4, space="PSUM") as ps2,
        ):
            in_dmas = []
            wk_t = constp.tile([C, WKC], BF16, tag="wk")
            in_dmas.append(nc.sync.dma_start(wk_t[:], wk_d.ap()))
            w1_t = wk_t[:, 0 : KW * M1].rearrange("c (kw m) -> c kw m", kw=KW)
            w2_t = wk_t[0:M1, KW * M1 :].rearrange("p (kh o) -> p kh o", kh=KH)

            xp_t = xinp.tile([C, RP, WP], BF16, tag="xp")
            in_dmas.append(nc.sync.dma_start(xp_t[:], xp_d.ap()))

            p2s_t = p2sp.tile([M1, RP, W], BF16)
            outsb_t = outsp.tile([C, H * W], F32)
            out_ap = out_d.ap()
            # 3 coarse out-DMAs (6 DMAs total <= 8 HWDGE queues, so no
            # same-queue WAW wait lands on any single-wait-slot DMA).
            dma_cuts = {2: (0, 3), 4: (3, 5), 6: (5, 7)}
            out_dmas = []

            # dummy matmul: first PE instruction, absorbs the wk-DMA wait so
            # every later matmul needs at most one new semaphore wait.
            dt = psd.tile([1, 1], F32, tag="dummy")
            nc.tensor.matmul(dt[:], wk_t[0:M1, 0:1], wk_t[0:M1, 0:1], start=True, stop=True)

            for k in range(NCH1):
                xt = xp_t[:, k * RCH : (k + 1) * RCH, :]
                pt = ps1.tile([M1, RCH, W], F32, tag="p1")
                for kw in range(KW):
                    nc.tensor.matmul(
                        pt[:],
                        w1_t[:, kw, :],
                        xt[:, :, DW * kw : DW * kw + W],
                        start=(kw == 0),
                        stop=(kw == KW - 1),
                    )
                nc.vector.tensor_copy(p2s_t[:, k * RCH : (k + 1) * RCH, :], pt[:])

            last_mm = None
            last_cp = None
            for j in range(NCH2):
                qt = ps2.tile([C, OCH, W], F32, tag="p2")
                for kh in range(KH):
                    r0 = j * OCH + DH * kh
                    last_mm = nc.tensor.matmul(
                        qt[:],
                        w2_t[:, kh, :],
                        p2s_t[:, r0 : r0 + OCH, :],
                        start=(kh == 0),
                        stop=(kh == KH - 1),
                    )
                last_cp = nc.vector.tensor_copy(
                    outsb_t[:, j * OCH * W : (j + 1) * OCH * W], qt[:]
                )
                if j in dma_cuts:
                    a, b = dma_cuts[j]
                    out_dmas.append(
                        nc.sync.dma_start(
                            out_ap[:, a * OCH * W : b * OCH * W],
                            outsb_t[:, a * OCH * W : b * OCH * W],
                        )
                    )

            # absorb each out-DMA completion into a chained SP nop so the
            # kernel-tail Drain (one wait slot per proc, few slots) only
            # needs engine semaphores, not per-DMA-queue ones.
            # absorb every proc's final tick into SP program order so the tail
            # Drain needs no (or one) semaphore wait in any schedule.
            for dep in in_dmas + out_dmas + [last_mm, last_cp]:
                nop = nc.sync.nop(nofuse=True)
                add_dep_helper(nop.ins, dep.ins, sync=True, reason="absorb tick")
    return nc


def _get_nc():
    global _NC
    if _NC is None:
        _NC = _build_nc()
    return _NC


def _prep_inputs(x, w3, w4, w5):
    w45 = (w5.astype(np.float64) @ w4.astype(np.float64)).astype(np.float32)
    # w1[c, kw, kh*CO+co] = w3[co, c, kh, kw]
    w1 = (
        np.transpose(w3, (1, 3, 2, 0))
        .reshape(C, KW, KH * CO)
        .astype(ml_dtypes.bfloat16)
    )
    # w2[p, kh, o] = w45[o, co] if p == kh*CO+co else 0  (zero rows kill the
    # blocks of p2s that belong to other kh taps in the K=80 contraction)
    w2 = np.zeros((M1, KH, C), np.float32)
    for kh in range(KH):
        w2[kh * CO : (kh + 1) * CO, kh, :] = w45.T
    wk = np.zeros((C, KW * M1 + KH * C), np.float32)
    wk[:, : KW * M1] = np.asarray(w1, np.float32).reshape(C, KW * M1)
    wk[:M1, KW * M1 :] = w2.reshape(M1, KH * C)
    wk = wk.astype(ml_dtypes.bfloat16)
    xp = np.zeros((N, C, RP, WP), np.float32)
    xp[:, :, PH : PH + H, PW : PW + W] = x
    xp = xp.astype(ml_dtypes.bfloat16)
    return xp, wk


def kernel(x, w3, w4, w5, trace=False):
    x = np.asarray(x, np.float32)
    w3 = np.asarray(w3, np.float32)
    w4 = np.asarray(w4, np.float32)
    w5 = np.asarray(w5, np.float32)
    xp, wk = _prep_inputs(x, w3, w4, w5)
    in_maps = [
        {"xp": np.ascontiguousarray(xp[n]), "wk": wk} for n in range(N)
    ]
    global _NC
    res = None
    last_err = None
    for attempt in range(6):
        if _NC is None:
            _NC = _build_nc(attempt)
        try:
            res = run_bass_kernel_spmd(
                _NC, in_maps, core_ids=list(range(N)), trace=trace
            )
            break
        except Exception as e:  # compile-schedule flake: rebuild perturbed
            last_err = e
            _NC = None
    if res is None:
        raise last_err
    out = np.stack(
        [np.asarray(res.results[n]["out"]).reshape(C, H, W) for n in range(N)]
    ).astype(np.float32)
    if trace:
        return out, res
    return out



# revision 20
# speedup vs baseline: 1.0551x; 1.0551x over previous
"""Trainium2 Bass kernel for dilated 5x7 conv (128->16ch) + 1x1 (16->16) + 1x1 (16->128).

Strategy (data-parallel, 1 image per core across 8 cores):
  reference: y = conv_dilated(x, w3, dil=(2,3), pad=(4,9)); y = w4@y; y = w5@y
  Host folds w45 = w5 @ w4  [128, 16].

  Per core, image x [128, 56, 56] column-padded to xpc [128, 56, 74] (bf16).
  Stage 1 (TensorE), REVERSED row order, chunks of 14,14,14,7,7 rows:
    for each kw in 0..6 one matmul, lhsT = w1[:, kw, :] [c=128, (kh,co)=80],
    rhs = xpc[:, rows, 3kw:3kw+56], PSUM-accumulating over kw
    -> P2[(kh,co), r, w]; only the 56 real rows are computed (pad rows of
    p2s are known-zero, memset once).  Evacuated PSUM->SBUF (f32->bf16) on
    ScalarE/Act so every downstream dep rides one engine-ordered semaphore.
  Shift-align (for a K=80 single-matmul stage 2):
    p2a[(kh,co), h, w] = p2s[(kh,co), h + 2kh, w].
    SBUF engines can't do per-partition-block row shifts and SBUF APs can't
    take diagonal partition steps, so shifted rows go through a DRAM
    scratch: 4 dump DMAs (SBUF->DRAM, plain APs) + 1 gather DMA per stage-2
    chunk (DRAM->SBUF, diagonal AP -- legal on the DRAM side).  Each gather
    lies inside ONE dump's range => single-dep consumers everywhere
    (Matmult fits ONE semaphore wait; lhsT deps ride on Ldweights).
  Stage 2 (TensorE): gathered chunks are ONE matmul each (K=80, M=128,
    lhsT=w2f[(kh,co),o]=w45[o,co]).  The hot rows [0,11) (dependent on the
    last stage-1 chunk) skip the shift: 5 PSUM-accumulating matmuls with
    per-kh zero-padded weights w2z reading p2s directly, so no DMA latency
    sits on the critical tail.
  Single-wait discipline: a matmul's own 2nd+ deps are absorbed by tiny
  preceding PE matmuls that naturally read a sliver of the same writer's
  region (the scheduler then subsumes the repeated dep); dummy/absorb
  matmuls write the hot-chunk PSUM bank (reused later via PE program
  order, costing no semaphore).
  Evacuate (VectorE, f32->bf16) and DMA out per chunk (bf16; host upcasts).
  PE p-state prewarm: dummy matmuls bridge the input-DMA wait so the tensor
  engine is at full clock when real work starts.
"""

import os
import sys

import numpy as np

for _p in ("/opt/trn_rl_repo", "/root/.axon_site/_ro/trn_rl_repo"):
    if os.path.isdir(_p) and _p not in sys.path:
        sys.path.insert(0, _p)

import ml_dtypes  # noqa: E402

import concourse.bass as bass  # noqa: E402
import concourse.tile as tile  # noqa: E402
from concourse.tile_rust import add_dep_helper  # noqa: E402
from concourse import mybir  # noqa: E402
from concourse.bass_utils import run_bass_kernel_spmd  # noqa: E402

N, C, H, W = 8, 128, 56, 56
CO = 16
KH, KW = 5, 7
DH, DW = 2, 3
PH, PW = 4, 9
WP = W + 2 * PW  # 74 padded cols
RP = H + 2 * PH  # 64 rows in p2s (incl. 4+4 known-zero pad rows)
M1 = KH * CO  # 80
# stage-1 chunks in x-row coords, processed in listed (reversed) order;
# 8 rows per chunk (matmul rhs row count > 8 trips an ISA limit).
S1_CHUNKS = [(48, 56), (40, 48), (32, 40), (24, 32), (16, 24), (8, 16), (0, 8)]
# x input DMAs (x-row ranges), first-needed first; chunk index of first use
X_DMAS = [(40, 56), (24, 40), (0, 24)]
X_FIRST_USE = {2: 1, 4: 2}  # chunk idx -> X_DMAS idx needing an absorb
# stage-2 chunks (out rows): gathered ones (1 PSUM bank each) + hot
Q_GATHER = [(47, 56), (38, 47), (29, 38), (20, 29), (12, 20)]
Q_HOT = (0, 12)
Q_HOT_SPLIT = [(0, 6), (6, 12)]  # matmul free size <= 512 per ISA
# dumps to DRAM scratch (p2s rows) + stage-1 chunk index they follow
DUMPS = [((44, 60), 1), ((36, 56), 2), ((28, 47), 3), ((12, 38), 5)]
# gather -> index of dump it reads from
G_DUMP = [0, 1, 2, 3, 3]
W1C = KW * M1  # 560
W2ZC = KH * C  # 640
WKC = W1C + W2ZC + C  # + dense w2f (128)
BF16 = mybir.dt.bfloat16
F32 = mybir.dt.float32

# prewarm tuning: plain dummies, then one wk-reading dummy, then more
# plain dummies bridging until the first x chunk is DMA-visible.
DUM_A = 4
DUM_B = 2

_NC = None


def _build_nc(attempt=0):
    nc = bass.Bass(
        "TRN2",
        target_bir_lowering=False,
        debug=False,
        enable_asserts=False,
        num_devices=N,
    )
    xpc_d = nc.dram_tensor("xpc", [C, H, WP], BF16, kind="ExternalInput")
    wk_d = nc.dram_tensor("wk", [C, WKC], BF16, kind="ExternalInput")
    scr_d = nc.dram_tensor("scr", [len(DUMPS), M1, RP, W], BF16, kind="ExternalInput")
    out_d = nc.dram_tensor("out", [C, H * W], BF16, kind="ExternalOutput")

    with tile.TileContext(nc) as tc:
        for _ in range(attempt):
            nc.sync.nop(nofuse=True)
        with (
            tc.tile_pool(name="const", bufs=1) as constp,
            tc.tile_pool(name="xin", bufs=1) as xinp,
            tc.tile_pool(name="p2s", bufs=1) as p2sp,
            tc.tile_pool(name="p2a", bufs=1) as p2ap,
            tc.tile_pool(name="outs", bufs=1) as outsp,
            tc.tile_pool(name="dum", bufs=1) as dump_,
            tc.tile_pool(name="ps1", bufs=2, space="PSUM") as ps1,
            tc.tile_pool(name="ps2", bufs=2, space="PSUM") as ps2g,
            tc.tile_pool(name="psh", bufs=2, space="PSUM") as psh,
        ):
            in_dmas = []
            aux_dmas = []
            out_dmas = []
            # dummy source for PE prewarm; memset first so dummies only
            # depend on one gpsimd op.
            dum_t = dump_.tile([C, 448], BF16, tag="dum")
            last_pool = nc.gpsimd.memset(dum_t[:], 0)

            wk_t = constp.tile([C, WKC], BF16, tag="wk")
            in_dmas.append(nc.gpsimd.dma_start(wk_t[:], wk_d.ap()))
            w1_t = wk_t[:, 0:W1C].rearrange("c (kw m) -> c kw m", kw=KW)
            w2z_t = wk_t[0:M1, W1C : W1C + W2ZC].rearrange(
                "p (kh o) -> p kh o", kh=KH
            )
            w2f_t = wk_t[0:M1, W1C + W2ZC :]  # [80, 128]

            xpc_t = xinp.tile([C, H, WP], BF16, tag="xpc")
            x_dmas = []
            for a, b in X_DMAS:
                d = nc.sync.dma_start(
                    xpc_t[:, a:b, :], xpc_d.ap()[:, a:b, :]
                )
                x_dmas.append(d)
                in_dmas.append(d)

            p2s_t = p2sp.tile([M1, RP, W], BF16)
            p2a_t = p2ap.tile([M1, H, W], BF16)
            outsb_t = outsp.tile([C, H * W], BF16)
            out_ap = out_d.ap()
            scr_ap = scr_d.ap()

            # p2s pad rows (sources outside [PH, PH+H)) are known zero
            nc.gpsimd.memset(p2s_t[:, 0:PH, :], 0)
            last_pool = nc.gpsimd.memset(p2s_t[:, PH + H : RP, :], 0)

            # dummy/absorb PSUM tile shares banks with the hot chunks via
            # pool rotation: later writers are ordered by PE program order
            # (no semaphore).
            hb = Q_HOT_SPLIT[0][1] - Q_HOT_SPLIT[0][0]
            dpt = psh.tile([C, hb, W], F32, tag="hot")

            def dummy_mm(rhs):
                return nc.tensor.matmul(
                    dpt[0:1, 0:1, 0:8].squeeze(1),
                    rhs[:, 0:1],
                    rhs[:, 0:8],
                    start=True,
                    stop=True,
                    skip_group_check=True,
                )

            # PE prewarm: keep the tensor engine continuously busy from the
            # end of the NEFF preamble until real data lands, so the p-state
            # ramp (1.2GHz for the first 3us of busy) completes on dummies.
            for _ in range(DUM_A):
                nc.tensor.matmul(
                    dpt[:, 0:6, :],
                    dum_t[:, 0:128],
                    dum_t[:, 0:336],
                    start=True,
                    stop=True,
                    skip_group_check=True,
                )
            # absorbs the wk-DMA wait into PE program order
            dummy_mm(wk_t)
            for _ in range(DUM_B):
                nc.tensor.matmul(
                    dpt[:, 0:6, :],
                    dum_t[:, 0:128],
                    dum_t[:, 0:336],
                    start=True,
                    stop=True,
                    skip_group_check=True,
                )

            def absorb(region_ap):
                """Tiny PE matmul naturally depending on region_ap's writer;
                the next matmul's repeated dep then subsumes to zero."""
                return dummy_mm(region_ap)

            def gather(qi):
                a, b = Q_GATHER[qi]
                src = scr_ap.copy()
                v = src.ap
                v.clear()
                v.extend(
                    [
                        [CO * RP * W + DH * W, KH],
                        [RP * W, CO],
                        [W, b - a],
                        [1, W],
                    ]
                )
                src.offset = G_DUMP[qi] * M1 * RP * W + a * W
                d = nc.sync.dma_start(p2a_t[:, a:b, :], src)
                aux_dmas.append(d)
                return d

            # stage 1 reversed; evacuate on ScalarE (Act)
            last_cast = None
            for ci, (a, b) in enumerate(S1_CHUNKS):
                ab = None
                if ci in X_FIRST_USE:
                    # chunk first touches a new x DMA region while also
                    # carrying a PSUM-reuse (Act) wait; absorb the DMA wait.
                    xa, _xb = X_DMAS[X_FIRST_USE[ci]]
                    ab = absorb(xpc_t[:, xa : xa + 1, :].squeeze(1))
                xt = xpc_t[:, a:b, :]
                pt = ps1.tile([M1, b - a, W], F32, tag="p1")
                for kw in range(KW):
                    mm = nc.tensor.matmul(
                        pt[:],
                        w1_t[:, kw, :],
                        xt[:, :, DW * kw : DW * kw + W],
                        start=(kw == 0),
                        stop=(kw == KW - 1),
                    )
                    if ab is not None and kw == 0:
                        add_dep_helper(mm.ins, ab.ins, sync=False, reason="order")
                last_cast = nc.scalar.copy(
                    p2s_t[:, PH + a : PH + b, :], pt[:]
                )
                for di, ((dlo, dhi), after) in enumerate(DUMPS):
                    if after == ci:
                        aux_dmas.append(
                            nc.gpsimd.dma_start(
                                scr_ap[di, :, dlo:dhi, :],
                                p2s_t[:, dlo:dhi, :],
                            )
                        )
                        for qi, gd in enumerate(G_DUMP):
                            if gd == di:
                                gather(qi)

            # stage 2: gathered chunks (single K=80 matmul each)
            last_mm = None
            last_cp = None
            s2_mms = []
            s2_cps = []

            def emit_out(a, b, qt):
                nonlocal last_cp
                last_cp = nc.vector.tensor_copy(
                    outsb_t[:, a * W : b * W], qt[:]
                )
                s2_cps.append(last_cp)

            def emit_out_dma(a, b):
                out_dmas.append(
                    nc.gpsimd.dma_start(
                        out_ap[:, a * W : b * W], outsb_t[:, a * W : b * W]
                    )
                )

            for qi, (a, b) in enumerate(Q_GATHER):
                ab = absorb(p2a_t[:, a : a + 1, :].squeeze(1))
                s2_mms.append(ab)
                qt = ps2g.tile([C, b - a, W], F32, tag="p2")
                last_mm = nc.tensor.matmul(
                    qt[:], w2f_t, p2a_t[:, a:b, :], start=True, stop=True
                )
                s2_mms.append(last_mm)
                add_dep_helper(last_mm.ins, ab.ins, sync=False, reason="order")
                emit_out(a, b, qt)
                if qi == 1:
                    emit_out_dma(38, 56)
                elif qi == 3:
                    emit_out_dma(20, 38)


            # hot chunks, multiplexed directly from p2s.  kh=2 first: its
            # rows avoid the memset pad region, so the group-opening matmul
            # carries only the Act (data) wait.
            kh_order = [2, 0, 1, 3, 4]
            for a, b in Q_HOT_SPLIT:
                qt = psh.tile([C, b - a, W], F32, tag="hot")
                for i, kh in enumerate(kh_order):
                    last_mm = nc.tensor.matmul(
                        qt[:],
                        w2z_t[:, kh, :],
                        p2s_t[:, a + DH * kh : b + DH * kh, :],
                        start=(i == 0),
                        stop=(i == len(kh_order) - 1),
                        skip_group_check=True,
                    )
                    s2_mms.append(last_mm)
                emit_out(a, b, qt)
            emit_out_dma(0, 20)

            # final PE dummy reads the last output copy's region: it waits
            # the DVE copy (importing DVE's vector clock into PE) and is
            # nosync-pinned after every stage-2 op, so PE's final tick
            # transitively implies all compute.  The SP absorb nops then
            # cover it plus the DMAs, leaving the tail Drain <= 1 wait.
            chain = dummy_mm(outsb_t[:, Q_HOT_SPLIT[-1][0] * W :])
            for m in s2_mms + s2_cps:
                add_dep_helper(chain.ins, m.ins, sync=False, reason="tail")
            for dep in (
                [chain]
                + in_dmas
                + aux_dmas
                + out_dmas
                + [last_cast, last_pool]
            ):
                nop = nc.sync.nop(nofuse=True)
                add_dep_helper(nop.ins, dep.ins, sync=True, reason="absorb tick")
    return nc


def _get_nc():
    global _NC
    if _NC is None:
        _NC = _build_nc()
    return _NC


def _prep_inputs(x, w3, w4, w5):
    w45 = (w5.astype(np.float64) @ w4.astype(np.float64)).astype(np.float32)
    # w1[c, kw, kh*CO+co] = w3[co, c, kh, kw]
    w1 = np.transpose(w3, (1, 3, 2, 0)).reshape(C, W1C)
    # w2z[p, kh, o] = w45[o, co] if p == kh*CO+co else 0
    w2z = np.zeros((M1, KH, C), np.float32)
    for kh in range(KH):
        w2z[kh * CO : (kh + 1) * CO, kh, :] = w45.T
    # dense w2f[kh*CO+co, o] = w45[o, co]
    w2f = np.tile(w45.T, (KH, 1))
    wk = np.zeros((C, WKC), np.float32)
    wk[:, :W1C] = w1
    wk[:M1, W1C : W1C + W2ZC] = w2z.reshape(M1, W2ZC)
    wk[:M1, W1C + W2ZC :] = w2f
    wk = wk.astype(ml_dtypes.bfloat16)
    xpc = np.zeros((N, C, H, WP), np.float32)
    xpc[:, :, :, PW : PW + W] = x
    xpc = xpc.astype(ml_dtypes.bfloat16)
    return xpc, wk


def kernel(x, w3, w4, w5, trace=False):
    x = np.asarray(x, np.float32)
    w3 = np.asarray(w3, np.float32)
    w4 = np.asarray(w4, np.float32)
    w5 = np.asarray(w5, np.float32)
    xpc, wk = _prep_inputs(x, w3, w4, w5)
    scr0 = np.zeros((len(DUMPS), M1, RP, W), ml_dtypes.bfloat16)
    in_maps = [
        {"xpc": np.ascontiguousarray(xpc[n]), "wk": wk, "scr": scr0}
        for n in range(N)
    ]
    global _NC
    res = None
    last_err = None
    for attempt in range(6):
        if _NC is None:
            _NC = _build_nc(attempt)
        try:
            res = run_bass_kernel_spmd(
                _NC, in_maps, core_ids=list(range(N)), trace=trace
            )
            break
        except Exception as e:  # compile-schedule flake: rebuild perturbed
            last_err = e
            _NC = None
    if res is None:
        raise last_err
    out = np.stack(
        [
            np.asarray(res.results[n]["out"])
            .astype(np.float32)
            .reshape(C, H, W)
            for n in range(N)
        ]
    )
    if trace:
        return out, res
    return out
